# revision 1
# baseline (speedup 1.0000x reference)
"""KAN-LSTM cell Trainium2 kernel.

Shapes (hardcoded): B=2048, I=256, H=512, K=32, D=I+H=768, 4 gates.
Sharding: pure data-parallel over batch across 8 cores (B_local=256).

Math per gate g:
  comb = [x | h_prev]                                   [B, D]
  A[b,d,k] = relu(comb[b,d]*W1[d,k] + b1[d,k])
  u[b,d]   = sum_k A*W2[d,k] + b2[d]
  gate     = u @ Wc + bc                                [B, H]
LSTM tail: f,i,o = sigmoid(g0,g1,g2); c~ = tanh(g3)
  c_t = f*c_prev + i*c~ ;  h_t = o*tanh(c_t)

Device formulation (feature-on-partition layout):
  relu(W1*x+b1)*W2 = W2*max(W1*x, -b1) + W2*b1
  -> per (gate,dtile,k): M = max(combT*W1col, -b1col)   (DVE tensor_scalar)
                         u += M*W2col                   (DVE scalar_tensor_tensor)
  The sum_k W2*b1 + b2 term is folded into the combiner bias on host:
     bc' = (sum_k W2*b1 + b2) @ Wc + bc
  Combiner: gate_T[h,b] = sum_dtiles Wc_tile.T @ u_tile (PE matmuls, PSUM acc)
  Tail computed in [h,b] layout.

All transposes use the DMA xbar engine (bf16 only): comb_T is bf16;
c_prev and the outputs cross the transpose as hi/lo bf16 pairs
(hi = bf16(v), lo = bf16(v - hi)) reconstructed by an fp32 add.
"""

import ml_dtypes
import numpy as np

import concourse.bacc as bacc
import concourse.bass as bass
import concourse.tile as tile
from concourse import mybir
from concourse.bass_utils import run_bass_kernel_spmd

# ---- problem constants ----
B, I, H, K = 2048, 256, 512, 32
D = I + H  # 768
G = 4
NCORES = 8
BL = B // NCORES          # 256 local batch
DT = D // 128             # 6 feature tiles
HT = H // 128             # 4 h tiles
BT = BL // 128            # 2 local-batch tiles
F32 = mybir.dt.float32
BF16 = mybir.dt.bfloat16
BF = ml_dtypes.bfloat16

_PROG_CACHE = {}


def _build_program():
    nc = bacc.Bacc(None, target_bir_lowering=False)

    # DRAM I/O (per-core shapes)
    xs = nc.dram_tensor("xs", [BL, I], BF16, kind="ExternalInput")
    hs = nc.dram_tensor("hs", [BL, H], BF16, kind="ExternalInput")
    cshi = nc.dram_tensor("cshi", [BL, H], BF16, kind="ExternalInput")
    cslo = nc.dram_tensor("cslo", [BL, H], BF16, kind="ExternalInput")
    w1 = nc.dram_tensor("w1", [128, G * DT * K], F32, kind="ExternalInput")
    nb1 = nc.dram_tensor("nb1", [128, G * DT * K], F32, kind="ExternalInput")
    w2 = nc.dram_tensor("w2", [128, G * DT * K], F32, kind="ExternalInput")
    wc = nc.dram_tensor("wc", [128, G * DT * H], F32, kind="ExternalInput")
    bcp = nc.dram_tensor("bcp", [128, G * HT], F32, kind="ExternalInput")
    ho = nc.dram_tensor("ho", [BL, H], F32, kind="ExternalOutput")
    co = nc.dram_tensor("co", [BL, H], F32, kind="ExternalOutput")

    with tile.TileContext(nc) as tc:
        with (
            tc.tile_pool(name="const", bufs=1) as const,
            tc.tile_pool(name="io", bufs=1) as io,
            tc.tile_pool(name="work", bufs=4) as work,
            tc.tile_pool(name="upool", bufs=1) as upool,
            tc.tile_pool(name="tail", bufs=1) as tailp,
            tc.tile_pool(name="gps", bufs=2, space="PSUM") as gps,
        ):
            # ---- load constants ----
            w1_sb = const.tile([128, G * DT * K], F32, tag="w1")
            nb1_sb = const.tile([128, G * DT * K], F32, tag="nb1")
            w2_sb = const.tile([128, G * DT * K], F32, tag="w2")
            wc_sb = const.tile([128, G * DT * H], F32, tag="wc")
            bcp_sb = const.tile([128, G * HT], F32, tag="bcp")
            for dst, src in [(w1_sb, w1), (nb1_sb, nb1), (w2_sb, w2),
                             (wc_sb, wc), (bcp_sb, bcp)]:
                nc.sync.dma_start(out=dst, in_=src[:, :])

            # ---- transpose in via DMA xbar: combT (bf16), cT (f32 hi+lo) ----
            combT = [io.tile([128, BL], BF16, tag=f"combT{t}", name=f"combT{t}")
                     for t in range(DT)]
            for t in range(I // 128):
                nc.sync.dma_start_transpose(combT[t], xs[:, t * 128:(t + 1) * 128])
            for j in range(HT):
                nc.sync.dma_start_transpose(combT[I // 128 + j],
                                            hs[:, j * 128:(j + 1) * 128])
            cT = []
            for j in range(HT):
                chi = io.tile([128, BL], BF16, tag=f"cthi{j}", name=f"cthi{j}")
                clo = io.tile([128, BL], BF16, tag=f"ctlo{j}", name=f"ctlo{j}")
                nc.sync.dma_start_transpose(chi, cshi[:, j * 128:(j + 1) * 128])
                nc.sync.dma_start_transpose(clo, cslo[:, j * 128:(j + 1) * 128])
                cfull = io.tile([128, BL], F32, tag=f"cT{j}", name=f"cT{j}")
                nc.vector.tensor_tensor(cfull, chi, clo, mybir.AluOpType.add)
                cT.append(cfull)

            # ---- stage 1: per-feature MLPs ----
            u_tiles = {}
            for g in range(G):
                for t in range(DT):
                    u = upool.tile([128, BL], F32, tag=f"u_{g}_{t}", name=f"u_{g}_{t}")
                    u_tiles[(g, t)] = u
                    base = g * DT * K + t * K
                    for k in range(K):
                        col = base + k
                        m = work.tile([128, BL], F32, tag="m")
                        nc.vector.tensor_scalar(
                            m, combT[t],
                            w1_sb[:, col:col + 1], nb1_sb[:, col:col + 1],
                            mybir.AluOpType.mult, mybir.AluOpType.max)
                        if k == 0:
                            nc.vector.tensor_scalar(
                                u, m, w2_sb[:, col:col + 1], None,
                                mybir.AluOpType.mult)
                        else:
                            nc.vector.scalar_tensor_tensor(
                                u, m, w2_sb[:, col:col + 1], u,
                                mybir.AluOpType.mult, mybir.AluOpType.add)

            # ---- combiner + LSTM tail, per h-tile; out via hi/lo xbar ----
            hhi_sb = io.tile([128, BT, H], BF16, tag="hhi_sb")
            hlo_sb = io.tile([128, BT, H], BF16, tag="hlo_sb")
            chi_sb = io.tile([128, BT, H], BF16, tag="chi_sb")
            clo_sb = io.tile([128, BT, H], BF16, tag="clo_sb")
            SIG = mybir.ActivationFunctionType.Sigmoid
            TANH = mybir.ActivationFunctionType.Tanh

            for hh in range(HT):
                ps = []
                for g in range(G):
                    p = gps.tile([128, BL], F32, tag=f"ps{g}", name=f"ps{g}")
                    ps.append(p)
                    for t in range(DT):
                        lcol = g * DT * H + t * H + hh * 128
                        nc.tensor.matmul(
                            p, wc_sb[:, lcol:lcol + 128], u_tiles[(g, t)],
                            start=(t == 0), stop=(t == DT - 1))
                f = tailp.tile([128, BL], F32, tag="f")
                i_ = tailp.tile([128, BL], F32, tag="i")
                o = tailp.tile([128, BL], F32, tag="o")
                cth = tailp.tile([128, BL], F32, tag="cth")
                for dst, src, g, fn in [(f, ps[0], 0, SIG), (i_, ps[1], 1, SIG),
                                        (o, ps[2], 2, SIG), (cth, ps[3], 3, TANH)]:
                    nc.scalar.activation(dst, src, fn,
                                         bias=bcp_sb[:, g * HT + hh:g * HT + hh + 1])
                t1 = tailp.tile([128, BL], F32, tag="t1")
                t2 = tailp.tile([128, BL], F32, tag="t2")
                ct = tailp.tile([128, BL], F32, tag="ct")
                tch = tailp.tile([128, BL], F32, tag="tch")
                ht = tailp.tile([128, BL], F32, tag="ht")
                nc.vector.tensor_tensor(t1, f, cT[hh], mybir.AluOpType.mult)
                nc.vector.tensor_tensor(t2, i_, cth, mybir.AluOpType.mult)
                nc.vector.tensor_tensor(ct, t1, t2, mybir.AluOpType.add)
                nc.scalar.activation(tch, ct, TANH)
                nc.vector.tensor_tensor(ht, o, tch, mybir.AluOpType.mult)

                # hi/lo split (device) then xbar transpose to natural layout
                for val, hi_dst, lo_dst in [(ht, hhi_sb, hlo_sb),
                                            (ct, chi_sb, clo_sb)]:
                    hi_b = tailp.tile([128, BL], BF16, tag="hi_b")
                    lo_f = tailp.tile([128, BL], F32, tag="lo_f")
                    lo_b = tailp.tile([128, BL], BF16, tag="lo_b")
                    nc.vector.tensor_copy(hi_b, val)
                    nc.vector.tensor_tensor(lo_f, val, hi_b,
                                            mybir.AluOpType.subtract)
                    nc.vector.tensor_copy(lo_b, lo_f)
                    for bt in range(BT):
                        nc.sync.dma_start_transpose(
                            hi_dst[:, bt, hh * 128:(hh + 1) * 128],
                            hi_b[:, bt * 128:(bt + 1) * 128])
                        nc.sync.dma_start_transpose(
                            lo_dst[:, bt, hh * 128:(hh + 1) * 128],
                            lo_b[:, bt * 128:(bt + 1) * 128])

            ho_sb = io.tile([128, BT, H], F32, tag="ho_sb")
            co_sb = io.tile([128, BT, H], F32, tag="co_sb")
            nc.vector.tensor_tensor(ho_sb, hhi_sb, hlo_sb, mybir.AluOpType.add)
            nc.vector.tensor_tensor(co_sb, chi_sb, clo_sb, mybir.AluOpType.add)
            nc.sync.dma_start(out=ho.rearrange("(t p) d -> p t d", p=128), in_=ho_sb)
            nc.sync.dma_start(out=co.rearrange("(t p) d -> p t d", p=128), in_=co_sb)

    nc.compile()
    return nc


def _host_prep(W1, b1, W2, b2, Wc, bc):
    """Rearrange weights into SBUF-ready layouts; fold the k-sum bias into bc."""
    # per-partition column layout: [128, G*DT*K], col (g,t,k) row p = val[g, t*128+p, k]
    def col_layout(w):  # w [G, D, K]
        return np.ascontiguousarray(
            w.reshape(G, DT, 128, K).transpose(2, 0, 1, 3).reshape(128, G * DT * K))

    w1h = col_layout(W1)
    nb1h = col_layout(-b1)
    w2h = col_layout(W2)
    # wc lhsT layout: [128, G*DT*H], col (g,t,h) row p = Wc[g, t*128+p, h]
    wch = np.ascontiguousarray(
        Wc.reshape(G, DT, 128, H).transpose(2, 0, 1, 3).reshape(128, G * DT * H))
    # folded bias: bc' = (sum_k W2*b1 + b2) @ Wc + bc   [G, H]
    Cd = (W2 * b1).sum(-1) + b2                       # [G, D]
    bcp = np.einsum('gd,gdh->gh', Cd, Wc) + bc        # [G, H]
    # [128, G*HT], col (g,hh) row p = bcp[g, hh*128+p]
    bcph = np.ascontiguousarray(
        bcp.reshape(G, HT, 128).transpose(2, 0, 1).reshape(128, G * HT))
    return w1h, nb1h, w2h, wch, bcph


def _make_in_maps(x, h_prev, c_prev, W1, b1, W2, b2, Wc, bc):
    x = np.asarray(x, np.float32)
    h_prev = np.asarray(h_prev, np.float32)
    c_prev = np.asarray(c_prev, np.float32)
    w1h, nb1h, w2h, wch, bcph = _host_prep(
        np.asarray(W1, np.float32), np.asarray(b1, np.float32),
        np.asarray(W2, np.float32), np.asarray(b2, np.float32),
        np.asarray(Wc, np.float32), np.asarray(bc, np.float32))
    xb = x.astype(BF)
    hb = h_prev.astype(BF)
    chi = c_prev.astype(BF)
    clo = (c_prev - chi.astype(np.float32)).astype(BF)

    in_maps = []
    for c in range(NCORES):
        sl = slice(c * BL, (c + 1) * BL)
        in_maps.append({
            "xs": np.ascontiguousarray(xb[sl]),
            "hs": np.ascontiguousarray(hb[sl]),
            "cshi": np.ascontiguousarray(chi[sl]),
            "cslo": np.ascontiguousarray(clo[sl]),
            "w1": w1h, "nb1": nb1h, "w2": w2h, "wc": wch, "bcp": bcph,
        })
    return in_maps


def kernel(x, h_prev, c_prev, W1, b1, W2, b2, Wc, bc):
    if "prog" not in _PROG_CACHE:
        _PROG_CACHE["prog"] = _build_program()
    nc = _PROG_CACHE["prog"]
    in_maps = _make_in_maps(x, h_prev, c_prev, W1, b1, W2, b2, Wc, bc)
    res = run_bass_kernel_spmd(nc, in_maps, core_ids=list(range(NCORES)))
    h_t = np.concatenate([res.results[c]["ho"] for c in range(NCORES)], axis=0)
    c_t = np.concatenate([res.results[c]["co"] for c in range(NCORES)], axis=0)
    return h_t, c_t



# revision 2
# speedup vs baseline: 2.5600x; 2.5600x over previous
"""KAN-LSTM cell Trainium2 kernel (v2: PE k-reduction).

Shapes (hardcoded): B=2048, I=256, H=512, K=32, D=I+H=768, 4 gates.
Sharding: pure data-parallel over batch across 8 cores (B_local=256).

Math per gate g:
  comb = [x | h_prev]                                   [B, D]
  A[b,d,k] = relu(comb[b,d]*W1[d,k] + b1[d,k])
  u[b,d]   = sum_k A*W2[d,k] + b2[d]
  gate     = u @ Wc + bc                                [B, H]
LSTM tail: f,i,o = sigmoid(g0,g1,g2); c~ = tanh(g3)
  c_t = f*c_prev + i*c~ ;  h_t = o*tanh(c_t)

Device formulation (all feature-on-partition, [*, batch] layout):
  relu(W1*x+b1)*W2 = W2*max(W1*x, -b1) + W2*b1
  The k-replicated comb layout puts 32 features x 4 k-slots on the 128
  partitions: rep[(f,kk), b] = comb[d0+f, b]  (DMA stride-0 broadcast).
  Activate (one DVE tensor_scalar, bf16 4x):  M = max(rep*W1, -b1)
  k-reduce on PE: u[32q:32q+32] += w2blk.T @ M  (8 k-chunk matmuls,
  PSUM accumulate, tile_position=(0,32q)).
  The sum_k W2*b1 + b2 term is folded into the combiner bias on host:
     bc' = (sum_k W2*b1 + b2) @ Wc + bc
  Combiner: gate[h,b] = sum_t Wc_tile.T @ u_tile (PE, PSUM acc), then
  sigmoid/tanh with bias read PSUM directly on ScalarE.
  Tail in [h, b] layout; host pre/post-transposes comb, c_prev, h_t, c_t
  (no device transposes at all).
"""

import ml_dtypes
import numpy as np

import concourse.bacc as bacc
import concourse.bass as bass
import concourse.tile as tile
from concourse import mybir
from concourse.bass_utils import run_bass_kernel_spmd

# ---- problem constants ----
B, I, H, K = 2048, 256, 512, 32
D = I + H  # 768
G = 4
NCORES = 8
BL = B // NCORES          # 256 local batch
DT = D // 128             # 6 feature tiles
HT = H // 128             # 4 h tiles
FG = 32                   # features per group
KC = 4                    # k per chunk (FG*KC = 128 partitions)
NKC = K // KC             # 8 k-chunks
NQ = 128 // FG            # 4 groups per dtile
NGRP = D // FG            # 24 rep tiles
NTILE = G * DT * NQ * NKC  # 768 activate tiles
F32 = mybir.dt.float32
BF16 = mybir.dt.bfloat16
BF = ml_dtypes.bfloat16

# tiles whose activate runs on ScalarE (relu path) instead of DVE (max
# path); chosen to balance engine load. Tile flat index:
#   idx = ((g*DT + t)*NQ + q)*NKC + kc
# Phase 1: none on ScalarE.
ACT_EVERY = 0  # if >0, every ACT_EVERY-th tile goes to ScalarE


def _is_act_tile(idx: int) -> bool:
    return ACT_EVERY > 0 and idx % ACT_EVERY == 0

_PROG_CACHE = {}


def _build_program():
    nc = bacc.Bacc(None, target_bir_lowering=False)

    # DRAM I/O (per-core shapes)
    combT = nc.dram_tensor("combT", [D, BL], BF16, kind="ExternalInput")
    ctT = nc.dram_tensor("ctT", [H, BL], F32, kind="ExternalInput")
    w1p = nc.dram_tensor("w1p", [128, NTILE], F32, kind="ExternalInput")
    nb1p = nc.dram_tensor("nb1p", [128, NTILE], F32, kind="ExternalInput")
    w2l = nc.dram_tensor("w2l", [128, NTILE * FG], BF16, kind="ExternalInput")
    wc = nc.dram_tensor("wc", [128, G * DT * H], BF16, kind="ExternalInput")
    bcp = nc.dram_tensor("bcp", [128, G * HT], F32, kind="ExternalInput")
    hoT = nc.dram_tensor("hoT", [H, BL], F32, kind="ExternalOutput")
    coT = nc.dram_tensor("coT", [H, BL], F32, kind="ExternalOutput")

    SIG = mybir.ActivationFunctionType.Sigmoid
    TANH = mybir.ActivationFunctionType.Tanh
    RELU = mybir.ActivationFunctionType.Relu

    with tile.TileContext(nc) as tc:
        with (
            tc.tile_pool(name="const", bufs=1) as const,
            tc.tile_pool(name="repp", bufs=1) as repp,
            tc.tile_pool(name="mp", bufs=12) as mp,
            tc.tile_pool(name="usb", bufs=1) as usb,
            tc.tile_pool(name="gsb", bufs=1) as gsb,
            tc.tile_pool(name="tailp", bufs=2) as tailp,
            tc.tile_pool(name="ups", bufs=3, space="PSUM") as ups,
            tc.tile_pool(name="gps", bufs=4, space="PSUM") as gps,
        ):
            # ---- constants ----
            w1p_sb = const.tile([128, NTILE], F32, tag="w1p")
            nb1p_sb = const.tile([128, NTILE], F32, tag="nb1p")
            w2l_sb = const.tile([128, NTILE * FG], BF16, tag="w2l")
            wc_sb = const.tile([128, G * DT * H], BF16, tag="wc")
            bcp_sb = const.tile([128, G * HT], F32, tag="bcp")
            nc.sync.dma_start(out=w1p_sb, in_=w1p[:, :])
            nc.sync.dma_start(out=nb1p_sb, in_=nb1p[:, :])
            nc.sync.dma_start(out=bcp_sb, in_=bcp[:, :])
            # big weight loads split per gate so early tiles unblock sooner
            gw = NTILE * FG // G
            for g in range(G):
                nc.sync.dma_start(out=w2l_sb[:, g * gw:(g + 1) * gw],
                                  in_=w2l[:, g * gw:(g + 1) * gw])
            cw = DT * H
            for g in range(G):
                nc.sync.dma_start(out=wc_sb[:, g * cw:(g + 1) * cw],
                                  in_=wc[:, g * cw:(g + 1) * cw])

            # c_prev tiles [128, BL] f32
            cT = []
            for j in range(HT):
                c = const.tile([128, BL], F32, tag=f"cT{j}", name=f"cT{j}")
                nc.sync.dma_start(out=c, in_=ctT[j * 128:(j + 1) * 128, :])
                cT.append(c)

            # ---- comb_rep tiles via stride-0 broadcast DMA ----
            # rep[(f,kk), b] = comb[grp*FG + f, b],  f in [0,FG), kk in [0,KC)
            rep = []
            for grp in range(NGRP):
                r = repp.tile([128, BL], BF16, tag=f"rep{grp}",
                              name=f"rep{grp}")
                d0 = grp * FG
                nc.sync.dma_start(
                    out=r,
                    in_=combT[d0:d0 + FG, None, :].to_broadcast((FG, KC, BL)))
                rep.append(r)

            # ---- stage 1 + combiner, per gate ----
            gates_sb = {}
            for g in range(G):
                for t in range(DT):
                    u_ps = ups.tile([128, BL], F32, tag="u", name=f"u_{g}_{t}")
                    for q in range(NQ):
                        for kc in range(NKC):
                            idx = ((g * DT + t) * NQ + q) * NKC + kc
                            m = mp.tile([128, BL], BF16, tag="m")
                            r = rep[t * NQ + q]
                            if _is_act_tile(idx):
                                # relu path: M = relu(rep*W1 + b1)
                                # (nb1p col negated on host for these)
                                nc.scalar.activation(
                                    m, r, RELU,
                                    bias=nb1p_sb[:, idx:idx + 1],
                                    scale=w1p_sb[:, idx:idx + 1])
                            else:
                                # max path: M = max(rep*W1, -b1)
                                nc.vector.tensor_scalar(
                                    m, r,
                                    w1p_sb[:, idx:idx + 1],
                                    nb1p_sb[:, idx:idx + 1],
                                    mybir.AluOpType.mult,
                                    mybir.AluOpType.max)
                            nc.tensor.matmul(
                                u_ps[FG * q:FG * (q + 1), :],
                                w2l_sb[:, idx * FG:(idx + 1) * FG],
                                m,
                                start=(kc == 0), stop=(kc == NKC - 1),
                                tile_position=(0, FG * q))
                    u_s = usb.tile([128, BL], BF16, tag=f"u_{g}_{t}",
                                   name=f"usb_{g}_{t}")
                    nc.vector.tensor_copy(u_s, u_ps)
                    usb_gt = u_s
                    if t == 0:
                        u_list = []
                    u_list.append(usb_gt)

                # combiner for gate g (needs all 6 u tiles)
                fn = TANH if g == 3 else SIG
                for hh in range(HT):
                    gp = gps.tile([128, BL], F32, tag="gp",
                                  name=f"gp_{g}_{hh}")
                    for t in range(DT):
                        lcol = g * DT * H + t * H + hh * 128
                        nc.tensor.matmul(gp, wc_sb[:, lcol:lcol + 128],
                                         u_list[t],
                                         start=(t == 0), stop=(t == DT - 1))
                    gs = gsb.tile([128, BL], F32, tag=f"g_{g}_{hh}",
                                  name=f"gate_{g}_{hh}")
                    col = g * HT + hh
                    nc.scalar.activation(gs, gp, fn,
                                         bias=bcp_sb[:, col:col + 1])
                    gates_sb[(g, hh)] = gs

            # ---- LSTM tail in [h, b] layout ----
            for hh in range(HT):
                f = gates_sb[(0, hh)]
                i_ = gates_sb[(1, hh)]
                o = gates_sb[(2, hh)]
                cth = gates_sb[(3, hh)]
                t1 = tailp.tile([128, BL], F32, tag="t1")
                t2 = tailp.tile([128, BL], F32, tag="t2")
                ct = tailp.tile([128, BL], F32, tag="ct", name=f"ct{hh}")
                tch = tailp.tile([128, BL], F32, tag="tch")
                ht = tailp.tile([128, BL], F32, tag="ht", name=f"ht{hh}")
                nc.vector.tensor_tensor(t1, f, cT[hh], mybir.AluOpType.mult)
                nc.vector.tensor_tensor(t2, i_, cth, mybir.AluOpType.mult)
                nc.vector.tensor_tensor(ct, t1, t2, mybir.AluOpType.add)
                nc.scalar.activation(tch, ct, TANH)
                nc.vector.tensor_tensor(ht, o, tch, mybir.AluOpType.mult)
                nc.sync.dma_start(out=coT[hh * 128:(hh + 1) * 128, :], in_=ct)
                nc.sync.dma_start(out=hoT[hh * 128:(hh + 1) * 128, :], in_=ht)

    nc.compile()
    return nc


def _host_prep(W1, b1, W2, b2, Wc, bc):
    """Rearrange weights into the tiled layouts described above."""
    # per-tile scalar columns: idx = ((g*DT+t)*NQ+q)*NKC+kc,
    # partition p = f*KC + kk -> feature d = t*128+q*FG+f, k = kc*KC+kk
    # w1x[g, d, k] laid out as [128, NTILE]
    w1r = W1.reshape(G, DT, NQ, FG, NKC, KC)          # g t q f kc kk
    b1r = b1.reshape(G, DT, NQ, FG, NKC, KC)
    w2r = W2.reshape(G, DT, NQ, FG, NKC, KC)
    # -> [f*KC+kk, g, t, q, kc] = [128, NTILE]
    def cols(a):
        # a: [G, DT, NQ, FG, NKC, KC] -> [FG*KC, G*DT*NQ*NKC]
        return np.ascontiguousarray(
            a.transpose(3, 5, 0, 1, 2, 4).reshape(FG * KC, NTILE))

    w1h = cols(w1r)
    nb1h = cols(-b1r)
    # ScalarE relu-path tiles need +b1 as bias instead of -b1
    if ACT_EVERY > 0:
        b1h = cols(b1r)
        for idx in range(NTILE):
            if _is_act_tile(idx):
                nb1h[:, idx] = b1h[:, idx]
    nb1h = np.ascontiguousarray(nb1h)

    # w2l: [128, NTILE*FG], block idx: [p=(f,kk), col f'] = W2 if f'==f
    w2blk = np.zeros((FG * KC, NTILE, FG), dtype=np.float32)
    w2cols = cols(w2r)                                 # [128, NTILE]
    fidx = (np.arange(FG * KC) // KC)                  # f of each partition
    for p in range(FG * KC):
        w2blk[p, :, fidx[p]] = w2cols[p, :]
    w2lh = np.ascontiguousarray(
        w2blk.reshape(FG * KC, NTILE * FG).astype(BF))

    # wc lhsT layout: [128, G*DT*H], col (g,t,h) row p = Wc[g, t*128+p, h]
    wch = np.ascontiguousarray(
        Wc.reshape(G, DT, 128, H).transpose(2, 0, 1, 3)
        .reshape(128, G * DT * H).astype(BF))

    # folded bias: bc' = (sum_k W2*b1 [max-path only] + b2) @ Wc + bc
    corr = W2 * b1                                    # [G, D, K]
    if ACT_EVERY > 0:
        corrr = corr.reshape(G, DT, NQ, FG, NKC, KC)
        for g in range(G):
            for t in range(DT):
                for q in range(NQ):
                    for kc in range(NKC):
                        idx = ((g * DT + t) * NQ + q) * NKC + kc
                        if _is_act_tile(idx):
                            corrr[g, t, q, :, kc, :] = 0.0
        corr = corrr.reshape(G, D, K)
    Cd = corr.sum(-1) + b2                            # [G, D]
    bcpv = np.einsum('gd,gdh->gh', Cd, Wc) + bc       # [G, H]
    bcph = np.ascontiguousarray(
        bcpv.reshape(G, HT, 128).transpose(2, 0, 1).reshape(128, G * HT))
    return w1h, nb1h, w2lh, wch, bcph


def _make_in_maps(x, h_prev, c_prev, W1, b1, W2, b2, Wc, bc):
    x = np.asarray(x, np.float32)
    h_prev = np.asarray(h_prev, np.float32)
    c_prev = np.asarray(c_prev, np.float32)
    w1h, nb1h, w2lh, wch, bcph = _host_prep(
        np.asarray(W1, np.float32), np.asarray(b1, np.float32),
        np.asarray(W2, np.float32), np.asarray(b2, np.float32),
        np.asarray(Wc, np.float32), np.asarray(bc, np.float32))
    combT_all = np.concatenate([x, h_prev], axis=1).T.astype(BF)  # [D, B]
    ctT_all = np.ascontiguousarray(c_prev.T)                      # [H, B]

    in_maps = []
    for c in range(NCORES):
        sl = slice(c * BL, (c + 1) * BL)
        in_maps.append({
            "combT": np.ascontiguousarray(combT_all[:, sl]),
            "ctT": np.ascontiguousarray(ctT_all[:, sl]),
            "w1p": w1h, "nb1p": nb1h, "w2l": w2lh, "wc": wch, "bcp": bcph,
        })
    return in_maps


def kernel(x, h_prev, c_prev, W1, b1, W2, b2, Wc, bc):
    if "prog" not in _PROG_CACHE:
        _PROG_CACHE["prog"] = _build_program()
    nc = _PROG_CACHE["prog"]
    in_maps = _make_in_maps(x, h_prev, c_prev, W1, b1, W2, b2, Wc, bc)
    res = run_bass_kernel_spmd(nc, in_maps, core_ids=list(range(NCORES)))
    h_t = np.concatenate(
        [res.results[c]["hoT"].T for c in range(NCORES)], axis=0)
    c_t = np.concatenate(
        [res.results[c]["coT"].T for c in range(NCORES)], axis=0)
    return h_t, c_t


# revision 4
# speedup vs baseline: 3.3262x; 1.2993x over previous
"""KAN-LSTM cell Trainium2 kernel (v2: PE k-reduction).

Shapes (hardcoded): B=2048, I=256, H=512, K=32, D=I+H=768, 4 gates.
Sharding: pure data-parallel over batch across 8 cores (B_local=256).

Math per gate g:
  comb = [x | h_prev]                                   [B, D]
  A[b,d,k] = relu(comb[b,d]*W1[d,k] + b1[d,k])
  u[b,d]   = sum_k A*W2[d,k] + b2[d]
  gate     = u @ Wc + bc                                [B, H]
LSTM tail: f,i,o = sigmoid(g0,g1,g2); c~ = tanh(g3)
  c_t = f*c_prev + i*c~ ;  h_t = o*tanh(c_t)

Device formulation (all feature-on-partition, [*, batch] layout):
  relu(W1*x+b1)*W2 = W2*max(W1*x, -b1) + W2*b1
  The k-replicated comb layout puts 32 features x 4 k-slots on the 128
  partitions: rep[(f,kk), b] = comb[d0+f, b]  (DMA stride-0 broadcast).
  Activate (one DVE tensor_scalar, bf16 4x):  M = max(rep*W1, -b1)
  k-reduce on PE: u[32q:32q+32] += w2blk.T @ M  (8 k-chunk matmuls,
  PSUM accumulate, tile_position=(0,32q)).
  The sum_k W2*b1 + b2 term is folded into the combiner bias on host:
     bc' = (sum_k W2*b1 + b2) @ Wc + bc
  Combiner: gate[h,b] = sum_t Wc_tile.T @ u_tile (PE, PSUM acc), then
  sigmoid/tanh with bias read PSUM directly on ScalarE.
  Tail in [h, b] layout; host pre/post-transposes comb, c_prev, h_t, c_t
  (no device transposes at all).
"""

import ml_dtypes
import numpy as np

import concourse.bacc as bacc
import concourse.bass as bass
import concourse.tile as tile
from concourse import mybir
from concourse.bass_utils import run_bass_kernel_spmd

# ---- problem constants ----
B, I, H, K = 2048, 256, 512, 32
D = I + H  # 768
G = 4
NCORES = 8
BL = B // NCORES          # 256 local batch
DT = D // 128             # 6 feature tiles
HT = H // 128             # 4 h tiles
FG = 32                   # features per group
KC = 4                    # k per chunk (FG*KC = 128 partitions)
NKC = K // KC             # 8 k-chunks
NQ = 128 // FG            # 4 groups per dtile
NGRP = D // FG            # 24 rep tiles
NTILE = G * DT * NQ * NKC  # 768 activate tiles
F32 = mybir.dt.float32
BF16 = mybir.dt.bfloat16
BF = ml_dtypes.bfloat16

# tiles whose activate runs on ScalarE (relu path) instead of DVE (max
# path); chosen to balance engine load (DVE ~256ns/tile, ScalarE
# ~491ns/tile -> ~1/3 of tiles on ScalarE). Tile flat index:
#   idx = ((g*DT + t)*NQ + q)*NKC + kc
ACT_MOD = 3  # idx % ACT_MOD == ACT_MOD-1 -> ScalarE


def _is_act_tile(idx: int) -> bool:
    return ACT_MOD > 0 and idx % ACT_MOD == ACT_MOD - 1

_PROG_CACHE = {}


def _build_program():
    nc = bacc.Bacc(None, target_bir_lowering=False)

    # DRAM I/O (per-core shapes)
    combT = nc.dram_tensor("combT", [D, BL], BF16, kind="ExternalInput")
    ctT = nc.dram_tensor("ctT", [H, BL], BF16, kind="ExternalInput")
    w1p = nc.dram_tensor("w1p", [128, NTILE], F32, kind="ExternalInput")
    nb1p = nc.dram_tensor("nb1p", [128, NTILE], F32, kind="ExternalInput")
    w2l = nc.dram_tensor("w2l", [128, NTILE * FG], BF16, kind="ExternalInput")
    wc = nc.dram_tensor("wc", [128, G * DT * H], BF16, kind="ExternalInput")
    bcp = nc.dram_tensor("bcp", [128, G * HT], F32, kind="ExternalInput")
    hoT = nc.dram_tensor("hoT", [H, BL], BF16, kind="ExternalOutput")
    coT = nc.dram_tensor("coT", [H, BL], BF16, kind="ExternalOutput")

    SIG = mybir.ActivationFunctionType.Sigmoid
    TANH = mybir.ActivationFunctionType.Tanh
    RELU = mybir.ActivationFunctionType.Relu

    with tile.TileContext(nc) as tc:
        with (
            tc.tile_pool(name="const", bufs=1) as const,
            tc.tile_pool(name="repp", bufs=1) as repp,
            tc.tile_pool(name="mp", bufs=12) as mp,
            tc.tile_pool(name="usb", bufs=1) as usb,
            tc.tile_pool(name="gsb", bufs=1) as gsb,
            tc.tile_pool(name="tailp", bufs=2) as tailp,
            tc.tile_pool(name="ups", bufs=3, space="PSUM") as ups,
            tc.tile_pool(name="gps", bufs=4, space="PSUM") as gps,
        ):
            # ---- constants ----
            w1p_sb = const.tile([128, NTILE], F32, tag="w1p")
            nb1p_sb = const.tile([128, NTILE], F32, tag="nb1p")
            w2l_sb = const.tile([128, NTILE * FG], BF16, tag="w2l")
            wc_sb = const.tile([128, G * DT * H], BF16, tag="wc")
            bcp_sb = const.tile([128, G * HT], F32, tag="bcp")
            nc.sync.dma_start(out=w1p_sb, in_=w1p[:, :])
            nc.sync.dma_start(out=nb1p_sb, in_=nb1p[:, :])
            nc.sync.dma_start(out=bcp_sb, in_=bcp[:, :])
            # big weight loads split per gate so early tiles unblock sooner
            gw = NTILE * FG // G
            for g in range(G):
                nc.sync.dma_start(out=w2l_sb[:, g * gw:(g + 1) * gw],
                                  in_=w2l[:, g * gw:(g + 1) * gw])
            cw = DT * H
            for g in range(G):
                nc.sync.dma_start(out=wc_sb[:, g * cw:(g + 1) * cw],
                                  in_=wc[:, g * cw:(g + 1) * cw])

            # c_prev tiles [128, BL] f32
            cT = []
            for j in range(HT):
                c = const.tile([128, BL], BF16, tag=f"cT{j}", name=f"cT{j}")
                nc.sync.dma_start(out=c, in_=ctT[j * 128:(j + 1) * 128, :])
                cT.append(c)

            # ---- comb_rep tiles via stride-0 broadcast DMA ----
            # rep[(f,kk), b] = comb[grp*FG + f, b],  f in [0,FG), kk in [0,KC)
            rep = []
            for grp in range(NGRP):
                r = repp.tile([128, BL], BF16, tag=f"rep{grp}",
                              name=f"rep{grp}")
                d0 = grp * FG
                nc.sync.dma_start(
                    out=r,
                    in_=combT[d0:d0 + FG, None, :].to_broadcast((FG, KC, BL)))
                rep.append(r)

            # ---- stage 1 + combiner, per gate ----
            gates_sb = {}
            for g in range(G):
                for t in range(DT):
                    u_ps = ups.tile([128, BL], F32, tag="u", name=f"u_{g}_{t}")
                    for q in range(NQ):
                        for kc in range(NKC):
                            idx = ((g * DT + t) * NQ + q) * NKC + kc
                            m = mp.tile([128, BL], BF16, tag="m")
                            r = rep[t * NQ + q]
                            if _is_act_tile(idx):
                                # relu path: M = relu(rep*W1 + b1)
                                # (nb1p col negated on host for these)
                                nc.scalar.activation(
                                    m, r, RELU,
                                    bias=nb1p_sb[:, idx:idx + 1],
                                    scale=w1p_sb[:, idx:idx + 1])
                            else:
                                # max path: M = max(rep*W1, -b1)
                                nc.vector.tensor_scalar(
                                    m, r,
                                    w1p_sb[:, idx:idx + 1],
                                    nb1p_sb[:, idx:idx + 1],
                                    mybir.AluOpType.mult,
                                    mybir.AluOpType.max)
                            nc.tensor.matmul(
                                u_ps[FG * q:FG * (q + 1), :],
                                w2l_sb[:, idx * FG:(idx + 1) * FG],
                                m,
                                start=(kc == 0), stop=(kc == NKC - 1),
                                tile_position=(0, FG * q))
                    u_s = usb.tile([128, BL], BF16, tag=f"u_{g}_{t}",
                                   name=f"usb_{g}_{t}")
                    nc.vector.tensor_copy(u_s, u_ps)
                    usb_gt = u_s
                    if t == 0:
                        u_list = []
                    u_list.append(usb_gt)

                # combiner for gate g (needs all 6 u tiles)
                fn = TANH if g == 3 else SIG
                for hh in range(HT):
                    gp = gps.tile([128, BL], F32, tag="gp",
                                  name=f"gp_{g}_{hh}")
                    for t in range(DT):
                        lcol = g * DT * H + t * H + hh * 128
                        nc.tensor.matmul(gp, wc_sb[:, lcol:lcol + 128],
                                         u_list[t],
                                         start=(t == 0), stop=(t == DT - 1))
                    gs = gsb.tile([128, BL], BF16, tag=f"g_{g}_{hh}",
                                  name=f"gate_{g}_{hh}")
                    col = g * HT + hh
                    nc.scalar.activation(gs, gp, fn,
                                         bias=bcp_sb[:, col:col + 1])
                    gates_sb[(g, hh)] = gs

            # ---- LSTM tail in [h, b] layout ----
            for hh in range(HT):
                f = gates_sb[(0, hh)]
                i_ = gates_sb[(1, hh)]
                o = gates_sb[(2, hh)]
                cth = gates_sb[(3, hh)]
                t1 = tailp.tile([128, BL], BF16, tag="t1")
                t2 = tailp.tile([128, BL], BF16, tag="t2")
                ct = tailp.tile([128, BL], BF16, tag="ct", name=f"ct{hh}")
                tch = tailp.tile([128, BL], BF16, tag="tch")
                ht = tailp.tile([128, BL], BF16, tag="ht", name=f"ht{hh}")
                nc.vector.tensor_tensor(t1, f, cT[hh], mybir.AluOpType.mult)
                nc.vector.tensor_tensor(t2, i_, cth, mybir.AluOpType.mult)
                nc.vector.tensor_tensor(ct, t1, t2, mybir.AluOpType.add)
                nc.scalar.activation(tch, ct, TANH)
                nc.vector.tensor_tensor(ht, o, tch, mybir.AluOpType.mult)
                nc.sync.dma_start(out=coT[hh * 128:(hh + 1) * 128, :], in_=ct)
                nc.sync.dma_start(out=hoT[hh * 128:(hh + 1) * 128, :], in_=ht)

    nc.compile()
    return nc


def _host_prep(W1, b1, W2, b2, Wc, bc):
    """Rearrange weights into the tiled layouts described above."""
    # per-tile scalar columns: idx = ((g*DT+t)*NQ+q)*NKC+kc,
    # partition p = f*KC + kk -> feature d = t*128+q*FG+f, k = kc*KC+kk
    # w1x[g, d, k] laid out as [128, NTILE]
    w1r = W1.reshape(G, DT, NQ, FG, NKC, KC)          # g t q f kc kk
    b1r = b1.reshape(G, DT, NQ, FG, NKC, KC)
    w2r = W2.reshape(G, DT, NQ, FG, NKC, KC)
    # -> [f*KC+kk, g, t, q, kc] = [128, NTILE]
    def cols(a):
        # a: [G, DT, NQ, FG, NKC, KC] -> [FG*KC, G*DT*NQ*NKC]
        return np.ascontiguousarray(
            a.transpose(3, 5, 0, 1, 2, 4).reshape(FG * KC, NTILE))

    w1h = cols(w1r)
    nb1h = cols(-b1r)
    # ScalarE relu-path tiles need +b1 as bias instead of -b1
    if ACT_MOD > 0:
        b1h = cols(b1r)
        for idx in range(NTILE):
            if _is_act_tile(idx):
                nb1h[:, idx] = b1h[:, idx]
    nb1h = np.ascontiguousarray(nb1h)

    # w2l: [128, NTILE*FG], block idx: [p=(f,kk), col f'] = W2 if f'==f
    w2blk = np.zeros((FG * KC, NTILE, FG), dtype=np.float32)
    w2cols = cols(w2r)                                 # [128, NTILE]
    fidx = (np.arange(FG * KC) // KC)                  # f of each partition
    for p in range(FG * KC):
        w2blk[p, :, fidx[p]] = w2cols[p, :]
    w2lh = np.ascontiguousarray(
        w2blk.reshape(FG * KC, NTILE * FG).astype(BF))

    # wc lhsT layout: [128, G*DT*H], col (g,t,h) row p = Wc[g, t*128+p, h]
    wch = np.ascontiguousarray(
        Wc.reshape(G, DT, 128, H).transpose(2, 0, 1, 3)
        .reshape(128, G * DT * H).astype(BF))

    # folded bias: bc' = (sum_k W2*b1 [max-path only] + b2) @ Wc + bc
    corr = W2 * b1                                    # [G, D, K]
    if ACT_MOD > 0:
        corrr = corr.reshape(G, DT, NQ, FG, NKC, KC)
        for g in range(G):
            for t in range(DT):
                for q in range(NQ):
                    for kc in range(NKC):
                        idx = ((g * DT + t) * NQ + q) * NKC + kc
                        if _is_act_tile(idx):
                            corrr[g, t, q, :, kc, :] = 0.0
        corr = corrr.reshape(G, D, K)
    Cd = corr.sum(-1) + b2                            # [G, D]
    bcpv = np.einsum('gd,gdh->gh', Cd, Wc) + bc       # [G, H]
    bcph = np.ascontiguousarray(
        bcpv.reshape(G, HT, 128).transpose(2, 0, 1).reshape(128, G * HT))
    return w1h, nb1h, w2lh, wch, bcph


def _make_in_maps(x, h_prev, c_prev, W1, b1, W2, b2, Wc, bc):
    x = np.asarray(x, np.float32)
    h_prev = np.asarray(h_prev, np.float32)
    c_prev = np.asarray(c_prev, np.float32)
    w1h, nb1h, w2lh, wch, bcph = _host_prep(
        np.asarray(W1, np.float32), np.asarray(b1, np.float32),
        np.asarray(W2, np.float32), np.asarray(b2, np.float32),
        np.asarray(Wc, np.float32), np.asarray(bc, np.float32))
    combT_all = np.concatenate([x, h_prev], axis=1).T.astype(BF)  # [D, B]
    ctT_all = c_prev.T.astype(BF)                                 # [H, B]

    in_maps = []
    for c in range(NCORES):
        sl = slice(c * BL, (c + 1) * BL)
        in_maps.append({
            "combT": np.ascontiguousarray(combT_all[:, sl]),
            "ctT": np.ascontiguousarray(ctT_all[:, sl]),
            "w1p": w1h, "nb1p": nb1h, "w2l": w2lh, "wc": wch, "bcp": bcph,
        })
    return in_maps


def kernel(x, h_prev, c_prev, W1, b1, W2, b2, Wc, bc):
    if "prog" not in _PROG_CACHE:
        _PROG_CACHE["prog"] = _build_program()
    nc = _PROG_CACHE["prog"]
    in_maps = _make_in_maps(x, h_prev, c_prev, W1, b1, W2, b2, Wc, bc)
    res = run_bass_kernel_spmd(nc, in_maps, core_ids=list(range(NCORES)))
    h_t = np.concatenate(
        [res.results[c]["hoT"].T.astype(np.float32) for c in range(NCORES)],
        axis=0)
    c_t = np.concatenate(
        [res.results[c]["coT"].T.astype(np.float32) for c in range(NCORES)],
        axis=0)
    return h_t, c_t


# revision 6
# speedup vs baseline: 3.6998x; 1.1123x over previous
"""KAN-LSTM cell Trainium2 kernel (v4: PE k-reduction + t-form activates).

Shapes (hardcoded): B=2048, I=256, H=512, K=32, D=I+H=768, 4 gates.
Sharding: pure data-parallel over batch across 8 cores (B_local=256).

Math per gate g:
  comb = [x | h_prev]                                   [B, D]
  A[b,d,k] = relu(comb[b,d]*W1[d,k] + b1[d,k])
  u[b,d]   = sum_k A*W2[d,k] + b2[d]
  gate     = u @ Wc + bc                                [B, H]
LSTM tail: f,i,o = sigmoid(g0,g1,g2); c~ = tanh(g3)
  c_t = f*c_prev + i*c~ ;  h_t = o*tanh(c_t)

Device formulation (all feature-on-partition, [*, batch] layout):
  The k-replicated comb layout puts 32 features x 4 k-slots on the 128
  partitions: rep[(f,kk), b] = comb[d0+f, b]  (DMA stride-0 broadcast).
  t-form: with t = -b1/W1 and coef = W2*|W1| (W1 clamped away from 0):
    W2*max(W1*c, -b1) = coef*max(c, t)            [W1>0]
                      = coef*max(c, t) + W2*W1*c  [W1<0]
    W2*relu(W1*c+b1)  = coef*relu(c - t)                      [W1>0]
                      = coef*relu(c - t) + W2*W1*c + W2*b1    [W1<0]
  so the activate is ONE single-scalar op per tile:
    DVE path:     M = max(rep, t)           (tensor_scalar, 1 PTR scalar)
    ScalarE path: M = relu(rep + (-t))      (activation, bias PTR, scale=1)
  k-reduce on PE: u[32q:32q+32] += w2l_blk.T @ M (8 k-chunk matmuls, PSUM
  accumulate, tile_position=(0,32q)); the linear residue lambda[d]*c[d]
  (lambda = sum_{k:W1<0} W2*W1) is seeded first as one diagonal matmul
  per (g,t): u = diag(lambda).T @ combT_tile.
  All constant residues (+ sum_k W2*b1 terms + b2) fold into the
  combiner bias on host: bc' = resid @ Wc + bc.
  Combiner: gate[h,b] = sum_t Wc_tile.T @ u_tile (PE, PSUM acc), then
  sigmoid/tanh with bias read PSUM directly on ScalarE.
  Tail in [h, b] layout, bf16; host pre/post-transposes comb, c_prev,
  h_t, c_t (no device transposes at all).
"""

import ml_dtypes
import numpy as np

import concourse.bacc as bacc
import concourse.bass as bass
import concourse.tile as tile
from concourse import mybir
from concourse.bass_utils import run_bass_kernel_spmd

# ---- problem constants ----
B, I, H, K = 2048, 256, 512, 32
D = I + H  # 768
G = 4
NCORES = 8
BL = B // NCORES          # 256 local batch
DT = D // 128             # 6 feature tiles
HT = H // 128             # 4 h tiles
FG = 32                   # features per group
KC = 4                    # k per chunk (FG*KC = 128 partitions)
NKC = K // KC             # 8 k-chunks
NQ = 128 // FG            # 4 groups per dtile
NGRP = D // FG            # 24 rep tiles
NTILE = G * DT * NQ * NKC  # 768 activate tiles
F32 = mybir.dt.float32
BF16 = mybir.dt.bfloat16
BF = ml_dtypes.bfloat16

# tiles whose activate runs on ScalarE (relu path) instead of DVE (max
# path); balance: DVE ~196ns/tile, ScalarE ~440ns/tile -> ~1/3 ScalarE.
ACT_MOD = 3  # idx % ACT_MOD == ACT_MOD-1 -> ScalarE


def _is_act_tile(idx: int) -> bool:
    return ACT_MOD > 0 and idx % ACT_MOD == ACT_MOD - 1

_PROG_CACHE = {}


def _build_program():
    nc = bacc.Bacc(None, target_bir_lowering=False)

    # DRAM I/O (per-core shapes)
    combT = nc.dram_tensor("combT", [D, BL], BF16, kind="ExternalInput")
    ctT = nc.dram_tensor("ctT", [H, BL], BF16, kind="ExternalInput")
    tcol = nc.dram_tensor("tcol", [128, NTILE], F32, kind="ExternalInput")
    w2l = nc.dram_tensor("w2l", [128, NTILE * FG], BF16, kind="ExternalInput")
    lam = nc.dram_tensor("lam", [128, G * DT * 128], BF16,
                         kind="ExternalInput")
    wc = nc.dram_tensor("wc", [128, G * DT * H], BF16, kind="ExternalInput")
    bcp = nc.dram_tensor("bcp", [128, G * HT], F32, kind="ExternalInput")
    hoT = nc.dram_tensor("hoT", [H, BL], BF16, kind="ExternalOutput")
    coT = nc.dram_tensor("coT", [H, BL], BF16, kind="ExternalOutput")

    SIG = mybir.ActivationFunctionType.Sigmoid
    TANH = mybir.ActivationFunctionType.Tanh
    RELU = mybir.ActivationFunctionType.Relu

    with tile.TileContext(nc) as tc:
        with (
            tc.tile_pool(name="const", bufs=1) as const,
            tc.tile_pool(name="repp", bufs=1) as repp,
            tc.tile_pool(name="mp", bufs=12) as mp,
            tc.tile_pool(name="usb", bufs=1) as usb,
            tc.tile_pool(name="gsb", bufs=1) as gsb,
            tc.tile_pool(name="tailp", bufs=2) as tailp,
            tc.tile_pool(name="ups", bufs=3, space="PSUM") as ups,
            tc.tile_pool(name="gps", bufs=4, space="PSUM") as gps,
        ):
            # ---- constants ----
            tcol_sb = const.tile([128, NTILE], F32, tag="tcol")
            w2l_sb = const.tile([128, NTILE * FG], BF16, tag="w2l")
            lam_sb = const.tile([128, G * DT * 128], BF16, tag="lam")
            wc_sb = const.tile([128, G * DT * H], BF16, tag="wc")
            bcp_sb = const.tile([128, G * HT], F32, tag="bcp")
            nc.sync.dma_start(out=tcol_sb, in_=tcol[:, :])
            nc.sync.dma_start(out=bcp_sb, in_=bcp[:, :])
            nc.sync.dma_start(out=lam_sb, in_=lam[:, :])
            # big weight loads split per gate so early tiles unblock sooner
            gw = NTILE * FG // G
            for g in range(G):
                nc.sync.dma_start(out=w2l_sb[:, g * gw:(g + 1) * gw],
                                  in_=w2l[:, g * gw:(g + 1) * gw])
            cw = DT * H
            for g in range(G):
                nc.sync.dma_start(out=wc_sb[:, g * cw:(g + 1) * cw],
                                  in_=wc[:, g * cw:(g + 1) * cw])

            # combT tiles (for the lambda passthrough) + c_prev tiles
            cbT = []
            for t in range(DT):
                cb = const.tile([128, BL], BF16, tag=f"cbT{t}",
                                name=f"cbT{t}")
                nc.sync.dma_start(out=cb, in_=combT[t * 128:(t + 1) * 128, :])
                cbT.append(cb)
            cT = []
            for j in range(HT):
                c = const.tile([128, BL], BF16, tag=f"cT{j}", name=f"cT{j}")
                nc.sync.dma_start(out=c, in_=ctT[j * 128:(j + 1) * 128, :])
                cT.append(c)

            # ---- comb_rep tiles via stride-0 broadcast DMA ----
            # rep[(f,kk), b] = comb[grp*FG + f, b],  f in [0,FG), kk in [0,KC)
            rep = []
            for grp in range(NGRP):
                r = repp.tile([128, BL], BF16, tag=f"rep{grp}",
                              name=f"rep{grp}")
                d0 = grp * FG
                nc.sync.dma_start(
                    out=r,
                    in_=combT[d0:d0 + FG, None, :].to_broadcast((FG, KC, BL)))
                rep.append(r)

            # ---- stage 1 + combiner, per gate ----
            gates_sb = {}
            for g in range(G):
                u_list = []
                for t in range(DT):
                    u_ps = ups.tile([128, BL], F32, tag="u", name=f"u_{g}_{t}")
                    # seed with the linear residue: u = diag(lambda) @ combT
                    lcol = (g * DT + t) * 128
                    nc.tensor.matmul(u_ps, lam_sb[:, lcol:lcol + 128],
                                     cbT[t], start=True, stop=False,
                                     skip_group_check=True)
                    for q in range(NQ):
                        for kc in range(NKC):
                            idx = ((g * DT + t) * NQ + q) * NKC + kc
                            r = rep[t * NQ + q]
                            if _is_act_tile(idx):
                                # M = relu(rep + bias), bias = -t
                                m = mp.tile([128, BL], BF16, tag="ms")
                                nc.scalar.activation(
                                    m, r, RELU,
                                    bias=tcol_sb[:, idx:idx + 1])
                            else:
                                # M = max(rep, t)
                                m = mp.tile([128, BL], BF16, tag="mv")
                                nc.vector.tensor_scalar(
                                    m, r, tcol_sb[:, idx:idx + 1], None,
                                    mybir.AluOpType.max)
                            nc.tensor.matmul(
                                u_ps[FG * q:FG * (q + 1), :],
                                w2l_sb[:, idx * FG:(idx + 1) * FG],
                                m,
                                start=False, stop=(kc == NKC - 1),
                                tile_position=(0, FG * q),
                                skip_group_check=True)
                    u_s = usb.tile([128, BL], BF16, tag=f"u_{g}_{t}",
                                   name=f"usb_{g}_{t}")
                    nc.vector.tensor_copy(u_s, u_ps)
                    u_list.append(u_s)

                # combiner for gate g (needs all 6 u tiles)
                fn = TANH if g == 3 else SIG
                for hh in range(HT):
                    gp = gps.tile([128, BL], F32, tag="gp",
                                  name=f"gp_{g}_{hh}")
                    for t in range(DT):
                        lcol = g * DT * H + t * H + hh * 128
                        nc.tensor.matmul(gp, wc_sb[:, lcol:lcol + 128],
                                         u_list[t],
                                         start=(t == 0), stop=(t == DT - 1))
                    gs = gsb.tile([128, BL], BF16, tag=f"g_{g}_{hh}",
                                  name=f"gate_{g}_{hh}")
                    col = g * HT + hh
                    nc.scalar.activation(gs, gp, fn,
                                         bias=bcp_sb[:, col:col + 1])
                    gates_sb[(g, hh)] = gs

            # ---- LSTM tail in [h, b] layout ----
            for hh in range(HT):
                f = gates_sb[(0, hh)]
                i_ = gates_sb[(1, hh)]
                o = gates_sb[(2, hh)]
                cth = gates_sb[(3, hh)]
                t1 = tailp.tile([128, BL], BF16, tag="t1")
                t2 = tailp.tile([128, BL], BF16, tag="t2")
                ct = tailp.tile([128, BL], BF16, tag="ct", name=f"ct{hh}")
                tch = tailp.tile([128, BL], BF16, tag="tch")
                ht = tailp.tile([128, BL], BF16, tag="ht", name=f"ht{hh}")
                nc.vector.tensor_tensor(t1, f, cT[hh], mybir.AluOpType.mult)
                nc.vector.tensor_tensor(t2, i_, cth, mybir.AluOpType.mult)
                nc.vector.tensor_tensor(ct, t1, t2, mybir.AluOpType.add)
                nc.scalar.activation(tch, ct, TANH)
                nc.vector.tensor_tensor(ht, o, tch, mybir.AluOpType.mult)
                nc.sync.dma_start(out=coT[hh * 128:(hh + 1) * 128, :], in_=ct)
                nc.sync.dma_start(out=hoT[hh * 128:(hh + 1) * 128, :], in_=ht)

    nc.compile()
    return nc


def _host_prep(W1, b1, W2, b2, Wc, bc):
    """Rearrange weights into the t-form tiled layouts."""
    eps = 1e-7
    W1s = np.where(np.abs(W1) < eps, np.where(W1 >= 0, eps, -eps), W1)
    neg = W1s < 0                                     # [G, D, K]
    coef = W2 * np.abs(W1s)                           # lhsT values, all pairs
    t = -b1 / W1s                                     # DVE max-path scalar

    # path mask per (g,d,k): ACT if its tile idx is an ACT tile
    # tile idx = ((g*DT+t)*NQ+q)*NKC+kc ; feature d = t*128+q*FG+f ;
    # k = kc*KC+kk
    gidx, didx, kidx = np.meshgrid(np.arange(G), np.arange(D), np.arange(K),
                                   indexing="ij")
    tt = didx // 128
    qq = (didx % 128) // FG
    kcc = kidx // KC
    tileidx = ((gidx * DT + tt) * NQ + qq) * NKC + kcc
    is_act = (tileidx % ACT_MOD == ACT_MOD - 1) if ACT_MOD > 0 \
        else np.zeros_like(tileidx, dtype=bool)

    # per-tile scalar column: DVE path: t ; ACT path: bias = -t = b1/W1
    scal = np.where(is_act, -t, t)                    # [G, D, K]

    # linear residue lambda[g,d] = sum_{k: W1<0} W2*W1  (both paths)
    lamv = np.where(neg, W2 * W1s, 0.0).sum(-1)       # [G, D]
    # constant residue:
    #  DVE path, W1>0: +W2*b1 ; ACT path, W1<0: +W2*b1 ; else 0
    constv = np.where(~is_act & ~neg, W2 * b1, 0.0).sum(-1) \
        + np.where(is_act & neg, W2 * b1, 0.0).sum(-1) + b2   # [G, D]

    def cols(a):  # [G, D, K] -> [FG*KC, NTILE] per-tile scalar columns
        ar = a.reshape(G, DT, NQ, FG, NKC, KC)
        return np.ascontiguousarray(
            ar.transpose(3, 5, 0, 1, 2, 4).reshape(FG * KC, NTILE))

    tcolh = cols(scal).astype(np.float32)

    # w2l: [128, NTILE*FG], block idx: [p=(f,kk), col f'] = coef if f'==f
    coefc = cols(coef)                                 # [128, NTILE]
    w2blk = np.zeros((FG * KC, NTILE, FG), dtype=np.float32)
    fidx = (np.arange(FG * KC) // KC)
    for p in range(FG * KC):
        w2blk[p, :, fidx[p]] = coefc[p, :]
    w2lh = np.ascontiguousarray(
        w2blk.reshape(FG * KC, NTILE * FG).astype(BF))

    # lam diag blocks: [128, G*DT*128], col (g,t,j) row p:
    #   lamv[g, t*128+p] if p==j else 0
    lamh = np.zeros((128, G * DT * 128), dtype=np.float32)
    lam_r = lamv.reshape(G, DT, 128)
    for g in range(G):
        for t in range(DT):
            base = (g * DT + t) * 128
            np.fill_diagonal(lamh[:, base:base + 128], lam_r[g, t])
    lamh = np.ascontiguousarray(lamh.astype(BF))

    # wc lhsT layout: [128, G*DT*H], col (g,t,h) row p = Wc[g, t*128+p, h]
    wch = np.ascontiguousarray(
        Wc.reshape(G, DT, 128, H).transpose(2, 0, 1, 3)
        .reshape(128, G * DT * H).astype(BF))

    # folded bias: bc' = constv @ Wc + bc
    bcpv = np.einsum('gd,gdh->gh', constv, Wc) + bc   # [G, H]
    bcph = np.ascontiguousarray(
        bcpv.reshape(G, HT, 128).transpose(2, 0, 1).reshape(128, G * HT))
    return tcolh, w2lh, lamh, wch, bcph


def _make_in_maps(x, h_prev, c_prev, W1, b1, W2, b2, Wc, bc):
    x = np.asarray(x, np.float32)
    h_prev = np.asarray(h_prev, np.float32)
    c_prev = np.asarray(c_prev, np.float32)
    tcolh, w2lh, lamh, wch, bcph = _host_prep(
        np.asarray(W1, np.float32), np.asarray(b1, np.float32),
        np.asarray(W2, np.float32), np.asarray(b2, np.float32),
        np.asarray(Wc, np.float32), np.asarray(bc, np.float32))
    combT_all = np.concatenate([x, h_prev], axis=1).T.astype(BF)  # [D, B]
    ctT_all = c_prev.T.astype(BF)                                 # [H, B]

    in_maps = []
    for c in range(NCORES):
        sl = slice(c * BL, (c + 1) * BL)
        in_maps.append({
            "combT": np.ascontiguousarray(combT_all[:, sl]),
            "ctT": np.ascontiguousarray(ctT_all[:, sl]),
            "tcol": tcolh, "w2l": w2lh, "lam": lamh, "wc": wch, "bcp": bcph,
        })
    return in_maps


def kernel(x, h_prev, c_prev, W1, b1, W2, b2, Wc, bc):
    if "prog" not in _PROG_CACHE:
        _PROG_CACHE["prog"] = _build_program()
    nc = _PROG_CACHE["prog"]
    in_maps = _make_in_maps(x, h_prev, c_prev, W1, b1, W2, b2, Wc, bc)
    res = run_bass_kernel_spmd(nc, in_maps, core_ids=list(range(NCORES)))
    h_t = np.concatenate(
        [res.results[c]["hoT"].T.astype(np.float32) for c in range(NCORES)],
        axis=0)
    c_t = np.concatenate(
        [res.results[c]["coT"].T.astype(np.float32) for c in range(NCORES)],
        axis=0)
    return h_t, c_t
